# revision 1
# baseline (speedup 1.0000x reference)
"""Trainium2 Bass kernel for nn_DeepGcnV2 (GCNII-style message passing).

Data-parallel over the batch of 32 graphs: 4 graphs per NeuronCore on 8
cores.  Per graph the device:
  - loads adj (fp32, pre-transposed on host so DRAM holds A^T row-major)
    with an on-the-fly cast to fp8e4 (0/1 values are exact in fp8),
  - computes degrees with a ones-vector matmul over A^T plus a PE-transpose
    "spray" to land them in per-partition layout,
  - runs proj -> 4x [LN -> relu -> W-tilde matmul -> normalized
    aggregation A @ v -> residual update] -> head.

Math identity used per layer (alpha=0.1, W~ = (1-beta) I + beta W):
  h' = h + s @ W~,  s = 0.9 * p + 0.1 * z,  p = Ahat z
     = h + 0.9 * dinv * (A @ (dinv * (z @ W~))) + (0.9 dinv^2 + 0.1) * (z @ W~)
so only one big (sparse-dense) matmul per layer, with A^T tiles stationary.
"""

import sys

for _p in ("/opt/trn_rl_repo",):
    if _p not in sys.path:
        sys.path.insert(0, _p)

import math

import ml_dtypes
import numpy as np

import concourse.bacc as bacc
import concourse.mybir as mybir
import concourse.tile as tile
from concourse import masks
from concourse.bass_types import AP
from concourse.bass_utils import run_bass_kernel_spmd

# Problem constants (hardcoded per contract)
BS, N, D, H, L = 32, 2048, 128, 64, 4
ALPHA, LAMDA, EPS = 0.1, 1.0, 1e-5
NCORES = 8
G = BS // NCORES          # graphs per core
P = 128                   # partitions
NT = N // P               # node tiles per graph
HALF = NT // 2            # tiles per psum half-batch

f32 = mybir.dt.float32
bf16 = mybir.dt.bfloat16
f8 = mybir.dt.float8e4


def _bmid(ap, n):
    """[128, F] AP -> [128, n, F] AP broadcast along an inserted middle dim."""
    dims = list(map(list, ap.ap))
    assert len(dims) == 2, dims
    return AP(ap.tensor, ap.offset, [dims[0], [0, n], dims[1]])


def _build_nc(skip_agg=False, skip_wpath=False, skip_ln=False, skip_deg=False, reps=1, graphs=G):
    nc = bacc.Bacc("TRN2", target_bir_lowering=False, debug=False)

    bt = nc.dram_tensor("bt", [G, N, N], f32, kind="ExternalInput").ap()
    xt = nc.dram_tensor("xt", [G, D, N], f32, kind="ExternalInput").ap()
    pw = nc.dram_tensor("pw", [D, H], f32, kind="ExternalInput").ap()
    pb = nc.dram_tensor("pb", [P, H], f32, kind="ExternalInput").ap()
    wt = nc.dram_tensor("wt", [L, H, H], bf16, kind="ExternalInput").ap()
    lng = nc.dram_tensor("lng", [L, P, H], f32, kind="ExternalInput").ap()
    lnb = nc.dram_tensor("lnb", [L, P, H], f32, kind="ExternalInput").ap()
    hw = nc.dram_tensor("hw", [P, H], f32, kind="ExternalInput").ap()
    hb = nc.dram_tensor("hb", [P, 1], f32, kind="ExternalInput").ap()
    out = nc.dram_tensor("out", [G, N, 1], f32, kind="ExternalOutput").ap()

    AX = mybir.AxisListType.X
    ADD = mybir.AluOpType.add
    MUL = mybir.AluOpType.mult
    SUB = mybir.AluOpType.subtract
    AF = mybir.ActivationFunctionType

    from contextlib import ExitStack
    with tile.TileContext(nc) as tc, ExitStack() as ctx:
        ep = ctx.enter_context
        cpool = ep(tc.tile_pool(name="const", bufs=1))
        prm = ep(tc.tile_pool(name="prm", bufs=1))
        at_pool = ep(tc.tile_pool(name="at", bufs=2))
        xt_pool = ep(tc.tile_pool(name="xts", bufs=2))
        h_pool = ep(tc.tile_pool(name="h", bufs=2))
        z_pool = ep(tc.tile_pool(name="z", bufs=2))
        zt_pool = ep(tc.tile_pool(name="zt", bufs=2))
        v_pool = ep(tc.tile_pool(name="v", bufs=2))
        tmp_pool = ep(tc.tile_pool(name="tmp", bufs=3))
        st_pool = ep(tc.tile_pool(name="st", bufs=4))
        dr_pool = ep(tc.tile_pool(name="dr", bufs=2))
        ob_pool = ep(tc.tile_pool(name="ob", bufs=2))
        ps_w = ep(tc.tile_pool(name="ps_w", bufs=2, space="PSUM"))
        ps_agg = ep(tc.tile_pool(name="ps_agg", bufs=2, space="PSUM"))
        ps_zt = ep(tc.tile_pool(name="ps_zt", bufs=1, space="PSUM"))
        ps_deg = ep(tc.tile_pool(name="ps_deg", bufs=1, space="PSUM"))
        if True:
            ident_bf = cpool.tile([P, P], bf16)
            masks.make_identity(nc, ident_bf[:])
            ident1 = cpool.tile([1, 1], f32)
            nc.vector.memset(ident1[:], 1.0)
            ones8 = cpool.tile([P, 1], f8)
            nc.vector.memset(ones8[:], 1.0)
            eps_sb = cpool.tile([P, 1], f32)
            nc.vector.memset(eps_sb[:], EPS)

            pw_sb = prm.tile([D, H], f32)
            nc.sync.dma_start(pw_sb[:], pw)
            pb_sb = prm.tile([P, H], f32)
            nc.sync.dma_start(pb_sb[:], pb)
            wt_sb = prm.tile([H, L, H], bf16)
            nc.sync.dma_start(wt_sb[:], wt.rearrange("l k m -> k l m"))
            lng_sb = prm.tile([P, L, H], f32)
            nc.sync.dma_start(lng_sb[:], lng.rearrange("l p m -> p l m"))
            lnb_sb = prm.tile([P, L, H], f32)
            nc.sync.dma_start(lnb_sb[:], lnb.rearrange("l p m -> p l m"))
            hw_sb = prm.tile([P, H], f32)
            nc.sync.dma_start(hw_sb[:], hw)
            hb_sb = prm.tile([P, 1], f32)
            nc.sync.dma_start(hb_sb[:], hb)

            for g in [gg for _ in range(reps) for gg in range(graphs)]:
                # ---- load A^T (cast fp32 -> fp8e4; 0/1 exact) and x^T ----
                at = at_pool.tile([P, NT, N], f8)
                for q in range(4):
                    nc.gpsimd.dma_start(
                        out=at[:, 4 * q:4 * q + 4, :],
                        in_=bt[g, 512 * q:512 * (q + 1), :].rearrange(
                            "(jb p) i -> p jb i", p=P),
                    )
                xts = xt_pool.tile([D, N], f32)
                nc.sync.dma_start(xts[:], xt[g])

                # ---- degrees: degT[1, i] = sum_j A^T[j, i] ----
                degrow = dr_pool.tile([1, N], f32)
                if skip_deg:
                    nc.vector.memset(degrow[:], 8.0)
                for c in range(0 if skip_deg else 4):
                    dps = ps_deg.tile([1, 512], f32, tag="degps")
                    for jb in range(NT):
                        nc.tensor.matmul(
                            dps[:, :], lhsT=ones8[:],
                            rhs=at[:, jb, 512 * c:512 * (c + 1)],
                            start=(jb == 0), stop=(jb == NT - 1))
                    nc.vector.tensor_copy(degrow[:, 512 * c:512 * (c + 1)], dps[:, :])
                # spray [1, N] -> [128, NT] via PE transpose of row slices
                spr = ps_deg.tile([P, NT], f32, tag="degps")
                for ib in range(NT):
                    nc.tensor.transpose(
                        spr[:, ib:ib + 1], degrow[0:1, ib * P:(ib + 1) * P], ident1[:])
                degc = st_pool.tile([P, NT], f32, tag="degc")
                nc.vector.tensor_copy(degc[:, :], spr[:, :])

                # dinv = 1/sqrt(deg_raw + 1); dinv9 = 0.9*dinv; c2 = 0.9*dinv^2+0.1
                sqd = st_pool.tile([P, NT], f32, tag="sqd")
                nc.scalar.activation(sqd[:], degc[:], AF.Sqrt, bias=1.0)
                dinv = st_pool.tile([P, NT], f32, tag="dinv")
                nc.vector.reciprocal(dinv[:], sqd[:])
                dinv9 = st_pool.tile([P, NT], f32, tag="dinv9")
                nc.vector.tensor_scalar(dinv9[:], dinv[:], 0.9, None, MUL)
                c2 = st_pool.tile([P, NT], f32, tag="c2")
                nc.vector.tensor_tensor(c2[:], dinv[:], dinv[:], op=MUL)
                nc.vector.tensor_scalar(c2[:], c2[:], 0.9, 0.1, MUL, ADD)

                # ---- h0 = x @ proj_w + proj_b ----
                h = h_pool.tile([P, NT, H], f32)
                for half in range(2):
                    hp = ps_w.tile([P, HALF, H], f32)
                    for q in range(HALF):
                        i = half * HALF + q
                        nc.tensor.matmul(
                            hp[:, q, :], lhsT=xts[:, i * P:(i + 1) * P],
                            rhs=pw_sb[:], start=True, stop=True)
                    nc.vector.tensor_tensor(
                        h[:, half * HALF:(half + 1) * HALF, :], hp[:, :, :],
                        _bmid(pb_sb[:, :], HALF), op=ADD)

                # ---- layers ----
                for l in range(L):
                    if skip_ln:
                        z = z_pool.tile([P, NT, H], bf16)
                        nc.scalar.activation(z[:, :, :], h[:, :, :], AF.Relu)
                    else:
                        musum = st_pool.tile([P, NT], f32, tag="musum")
                        nc.vector.tensor_reduce(musum[:], h[:, :, :], axis=AX, op=ADD)
                        sqh = tmp_pool.tile([P, NT, H], f32, tag="big")
                        nc.scalar.activation(sqh[:], h[:, :, :], AF.Square)
                        ssq = st_pool.tile([P, NT], f32, tag="ssq")
                        nc.vector.tensor_reduce(ssq[:], sqh[:, :, :], axis=AX, op=ADD)
                        mu = st_pool.tile([P, NT], f32, tag="mu")
                        nc.vector.tensor_scalar(mu[:], musum[:], 1.0 / H, None, MUL)
                        var = st_pool.tile([P, NT], f32, tag="var")
                        nc.vector.tensor_tensor(var[:], mu[:], mu[:], op=MUL)
                        nc.vector.tensor_scalar(ssq[:], ssq[:], 1.0 / H, None, MUL)
                        nc.vector.tensor_tensor(var[:], ssq[:], var[:], op=SUB)
                        stdv = st_pool.tile([P, NT], f32, tag="stdv")
                        nc.scalar.activation(stdv[:], var[:], AF.Sqrt, bias=eps_sb[:, 0:1])
                        rstd = st_pool.tile([P, NT], f32, tag="rstd")
                        nc.vector.reciprocal(rstd[:], stdv[:])

                        # z = relu(((h - mu) * rstd) * g + b)  (bf16)
                        zf = tmp_pool.tile([P, NT, H], f32, tag="big")
                        nc.vector.tensor_tensor(
                            zf[:, :, :], h[:, :, :], mu[:].broadcast_to([P, NT, H]), op=SUB)
                        nc.vector.tensor_tensor(
                            zf[:, :, :], zf[:, :, :], rstd[:].broadcast_to([P, NT, H]), op=MUL)
                        nc.vector.tensor_tensor(
                            zf[:, :, :], zf[:, :, :], _bmid(lng_sb[:, l, :], NT), op=MUL)
                        nc.vector.tensor_tensor(
                            zf[:, :, :], zf[:, :, :], _bmid(lnb_sb[:, l, :], NT), op=ADD)
                        z = z_pool.tile([P, NT, H], bf16)
                        nc.scalar.activation(z[:, :, :], zf[:, :, :], AF.Relu)

                    # z^T via PE transposes
                    zts = zt_pool.tile([H, N], bf16)
                    for half in range(0 if skip_wpath else 2):
                        zp = ps_zt.tile([H, HALF, P], bf16)
                        for q in range(HALF):
                            i = half * HALF + q
                            nc.tensor.transpose(zp[:, q, :], z[:, i, :], ident_bf[:])
                        nc.vector.tensor_copy(
                            zts[:, half * HALF * P:(half + 1) * HALF * P], zp[:, :, :])

                    # w = z @ W~;  v = dinv * w;  h += c2 * w
                    v = v_pool.tile([P, NT, H], bf16)
                    if skip_wpath:
                        nc.vector.tensor_copy(v[:, :, :], z[:, :, :])
                    for half in range(0 if skip_wpath else 2):
                        wp = ps_w.tile([P, HALF, H], f32)
                        for q in range(HALF):
                            j = half * HALF + q
                            nc.tensor.matmul(
                                wp[:, q, :], lhsT=zts[:, j * P:(j + 1) * P],
                                rhs=wt_sb[:, l, :], start=True, stop=True)
                        hs = slice(half * HALF, (half + 1) * HALF)
                        nc.vector.tensor_tensor(
                            v[:, hs, :], wp[:, :, :],
                            dinv[:, hs].broadcast_to([P, HALF, H]), op=MUL)
                        cw = tmp_pool.tile([P, HALF, H], f32, tag="half")
                        nc.vector.tensor_tensor(
                            cw[:, :, :], wp[:, :, :],
                            c2[:, hs].broadcast_to([P, HALF, H]), op=MUL)
                        nc.vector.tensor_tensor(
                            h[:, hs, :], h[:, hs, :], cw[:, :, :], op=ADD)

                    # agg = A @ v  (A^T tiles stationary, fp8);  h += dinv9 * agg
                    for half in range(0 if skip_agg else 2):
                        ap_ = ps_agg.tile([P, HALF, H], f32)
                        for q in range(HALF):
                            i = half * HALF + q
                            for jb in range(NT):
                                nc.tensor.matmul(
                                    ap_[:, q, :],
                                    lhsT=at[:, jb, i * P:(i + 1) * P],
                                    rhs=v[:, jb, :],
                                    start=(jb == 0), stop=(jb == NT - 1))
                        hs = slice(half * HALF, (half + 1) * HALF)
                        ag = tmp_pool.tile([P, HALF, H], f32, tag="half")
                        nc.vector.tensor_tensor(
                            ag[:, :, :], ap_[:, :, :],
                            dinv9[:, hs].broadcast_to([P, HALF, H]), op=MUL)
                        nc.vector.tensor_tensor(
                            h[:, hs, :], h[:, hs, :], ag[:, :, :], op=ADD)

                # ---- head: out = h @ head_w + head_b ----
                th = tmp_pool.tile([P, NT, H], f32, tag="big")
                nc.vector.tensor_tensor(
                    th[:, :, :], h[:, :, :], _bmid(hw_sb[:, :], NT), op=MUL)
                osb = ob_pool.tile([P, NT], f32)
                nc.vector.tensor_reduce(osb[:], th[:, :, :], axis=AX, op=ADD)
                nc.vector.tensor_scalar(osb[:], osb[:], hb_sb[:, 0:1], None, ADD)
                nc.sync.dma_start(
                    out=out[g].rearrange("(ib p) one -> p (ib one)", p=P),
                    in_=osb[:, :])

    nc.compile()
    return nc


_NC = None


def _get_nc():
    global _NC
    if _NC is None:
        _NC = _build_nc()
    return _NC


def _prep_in_maps(inputs):
    x = np.asarray(inputs["x"], np.float32)
    adj = np.asarray(inputs["adj"], np.float32)
    proj_w = np.asarray(inputs["proj_w"], np.float32)
    proj_b = np.asarray(inputs["proj_b"], np.float32)
    ln_g = np.asarray(inputs["ln_g"], np.float32)
    ln_b = np.asarray(inputs["ln_b"], np.float32)
    conv_w = np.asarray(inputs["conv_w"], np.float32)
    head_w = np.asarray(inputs["head_w"], np.float32)
    head_b = np.asarray(inputs["head_b"], np.float32)

    wtilde = np.empty((L, H, H), np.float32)
    for l in range(L):
        beta = math.log(LAMDA / (l + 1) + 1.0)
        wtilde[l] = (1.0 - beta) * np.eye(H, dtype=np.float32) + beta * conv_w[l]

    shared = {
        "pw": np.ascontiguousarray(proj_w),
        "pb": np.ascontiguousarray(np.broadcast_to(proj_b[None, :], (P, H))),
        "wt": wtilde.astype(ml_dtypes.bfloat16),
        "lng": np.ascontiguousarray(np.broadcast_to(ln_g[:, None, :], (L, P, H))),
        "lnb": np.ascontiguousarray(np.broadcast_to(ln_b[:, None, :], (L, P, H))),
        "hw": np.ascontiguousarray(np.broadcast_to(head_w[:, 0][None, :], (P, H))),
        "hb": np.full((P, 1), float(head_b[0]), np.float32),
    }
    in_maps = []
    for c in range(NCORES):
        sl = slice(c * G, (c + 1) * G)
        in_maps.append(dict(
            shared,
            bt=np.ascontiguousarray(adj[sl].transpose(0, 2, 1)),
            xt=np.ascontiguousarray(x[sl].transpose(0, 2, 1)),
        ))
    return in_maps


def kernel(**inputs) -> np.ndarray:
    nc = _get_nc()
    in_maps = _prep_in_maps(inputs)
    res = run_bass_kernel_spmd(nc, in_maps, list(range(NCORES)))
    return np.concatenate([res.results[c]["out"] for c in range(NCORES)], axis=0)



# revision 11
# speedup vs baseline: 34.8601x; 34.8601x over previous
"""Trainium2 Bass kernel for nn_DeepGcnV2 (GCNII-style message passing).

Data-parallel over the batch of 32 graphs: 4 graphs per NeuronCore on 8
cores.  Host prep casts A^T to fp8 (0/1 exact) and precomputes the
degree-derived per-node scalings, so the device only does the dense
linear algebra:

  proj -> 4x [ LN(bn_stats) -> relu -> aggregation into a transposed
  PSUM accumulator -> fused W-tilde matmul -> residual update ] -> head

Per layer the aggregation computes X^T in PSUM [128=2x64 feat rows, i]:
  rows 0:64   S0 = sum_{jb even} z'[jb]^T A^T[jb, :]  (+ self terms)
  rows 64:128 S1 = sum_{jb odd}  z'[jb]^T A^T[jb, :]
using z' tiles as the stationary operand (no transposes needed) and fp8
A^T as the 512-wide moving operand; even/odd tiles land in disjoint PE
column groups so they execute concurrently.  The self/initial-residual
term (sc2 * z') is accumulated into S0 via identity-rhs matmuls.  Then
  h += dinv9 * (X^T.T @ [W~; W~])
folds the neighbor, self-loop, and initial-residual contributions plus
the GCNII weight transform into a single 128-contraction matmul.

Math identity (alpha=0.1, W~ = (1-beta) I + beta W):
  h' = h + 0.9*dinv*(A @ (dinv*z)) @ W~ + (0.9*dinv^2+0.1)*(z @ W~)
     = h + dinv9 * ((agg + sc2*z') @ W~),  sc2 = 1 + (deg+1)/9
"""

import sys

for _p in ("/opt/trn_rl_repo",):
    if _p not in sys.path:
        sys.path.insert(0, _p)

import math

import ml_dtypes
import numpy as np

import concourse.bacc as bacc
import concourse.mybir as mybir
import concourse.tile as tile
from concourse import masks
from concourse.bass_types import AP
from concourse.bass_utils import run_bass_kernel_spmd

# Problem constants (hardcoded per contract)
BS, N, D, H, L = 32, 2048, 128, 64, 4
ALPHA, LAMDA, EPS = 0.1, 1.0, 1e-5
NCORES = 8
G = BS // NCORES          # graphs per core
P = 128                   # partitions
NT = N // P               # node tiles per graph (16)
NB = N // 512             # 512-wide aggregation blocks (4)
HALF = NT // 2            # tiles per psum half-batch (8)

f32 = mybir.dt.float32
bf16 = mybir.dt.bfloat16
f8 = mybir.dt.float8e4


def _bmid(ap, n):
    """[128, F] AP -> [128, n, F] AP broadcast along an inserted middle dim."""
    dims = list(map(list, ap.ap))
    assert len(dims) == 2, dims
    return AP(ap.tensor, ap.offset, [dims[0], [0, n], dims[1]])


def _build_nc(general_ln=False):
    nc = bacc.Bacc("TRN2", target_bir_lowering=False, debug=False)

    bt = nc.dram_tensor("bt", [G, N, N], f8, kind="ExternalInput").ap()
    xt = nc.dram_tensor("xt", [G, D, N], bf16, kind="ExternalInput").ap()
    pw = nc.dram_tensor("pw", [D, H], bf16, kind="ExternalInput").ap()
    pb = nc.dram_tensor("pb", [P, H], f32, kind="ExternalInput").ap()
    w2 = nc.dram_tensor("w2", [L, P, H], bf16, kind="ExternalInput").ap()
    lng = nc.dram_tensor("lng", [L, P, H], f32, kind="ExternalInput").ap()
    lnb = nc.dram_tensor("lnb", [L, P, H], f32, kind="ExternalInput").ap()
    hw = nc.dram_tensor("hw", [P, H], f32, kind="ExternalInput").ap()
    hb = nc.dram_tensor("hb", [P, 1], f32, kind="ExternalInput").ap()
    dv = nc.dram_tensor("dv", [G, P, NT], f32, kind="ExternalInput").ap()
    dv9 = nc.dram_tensor("dv9", [G, P, NT], f32, kind="ExternalInput").ap()
    sc2 = nc.dram_tensor("sc2", [G, P, NT], f32, kind="ExternalInput").ap()
    out = nc.dram_tensor("out", [G, N, 1], f32, kind="ExternalOutput").ap()

    AX = mybir.AxisListType.X
    ADD = mybir.AluOpType.add
    MUL = mybir.AluOpType.mult
    SUB = mybir.AluOpType.subtract
    AF = mybir.ActivationFunctionType

    from contextlib import ExitStack
    with tile.TileContext(nc) as tc, ExitStack() as ctx:
        ep = ctx.enter_context
        cpool = ep(tc.tile_pool(name="const", bufs=1))
        prm = ep(tc.tile_pool(name="prm", bufs=1))
        at_pool = ep(tc.tile_pool(name="at", bufs=3))
        xt_pool = ep(tc.tile_pool(name="xts", bufs=2))
        h_pool = ep(tc.tile_pool(name="h", bufs=2))
        z_pool = ep(tc.tile_pool(name="z", bufs=2))
        scz_pool = ep(tc.tile_pool(name="scz", bufs=2))
        xsb_pool = ep(tc.tile_pool(name="xsb", bufs=2))
        st_pool = ep(tc.tile_pool(name="st", bufs=2))
        tmp_pool = ep(tc.tile_pool(name="tmp", bufs=2))
        ob_pool = ep(tc.tile_pool(name="ob", bufs=2))
        ps_x = ep(tc.tile_pool(name="ps_x", bufs=4, space="PSUM"))
        ps_h = ep(tc.tile_pool(name="ps_h", bufs=2, space="PSUM"))

        ident_bf = cpool.tile([P, P], bf16)
        masks.make_identity(nc, ident_bf[:])
        eps_sb = cpool.tile([P, 1], f32)
        nc.vector.memset(eps_sb[:], EPS)

        pw_sb = prm.tile([D, H], bf16)
        nc.sync.dma_start(pw_sb[:], pw)
        pb_sb = prm.tile([P, H], f32)
        nc.sync.dma_start(pb_sb[:], pb)
        w2_sb = prm.tile([P, L, H], bf16)
        nc.sync.dma_start(w2_sb[:], w2.rearrange("l p m -> p l m"))
        hw_sb = prm.tile([P, H], f32)
        nc.sync.dma_start(hw_sb[:], hw)
        hb_sb = prm.tile([P, 1], f32)
        nc.sync.dma_start(hb_sb[:], hb)
        dv_sb = prm.tile([P, G, NT], f32)
        nc.sync.dma_start(dv_sb[:], dv.rearrange("g p t -> p g t"))
        dv9_sb = prm.tile([P, G, NT], f32)
        nc.sync.dma_start(dv9_sb[:], dv9.rearrange("g p t -> p g t"))
        sc2_sb = prm.tile([P, G, NT], f32)
        nc.sync.dma_start(sc2_sb[:], sc2.rearrange("g p t -> p g t"))
        if general_ln:
            lng_sb = prm.tile([P, L, H], f32)
            nc.sync.dma_start(lng_sb[:], lng.rearrange("l p m -> p l m"))
            lnb_sb = prm.tile([P, L, H], f32)
            nc.sync.dma_start(lnb_sb[:], lnb.rearrange("l p m -> p l m"))

        ats = {}
        xts_t = {}

        def load_graph(g):
            ats[g] = at_pool.tile([P, NT, N], f8, name="at", tag="at")
            nc.sync.dma_start(
                ats[g][:, :, :], bt[g].rearrange("(jb p) i -> p jb i", p=P))
            xts_t[g] = xt_pool.tile([D, N], bf16, name="xts", tag="xts")
            nc.scalar.dma_start(xts_t[g][:], xt[g])

        def proj(g, h):
            xts = xts_t[g]
            for half in range(2):
                hp = ps_h.tile([P, HALF, H], f32, tag="hps")
                for q in range(HALF):
                    i = half * HALF + q
                    nc.tensor.matmul(
                        hp[:, q, :], lhsT=xts[:, i * P:(i + 1) * P],
                        rhs=pw_sb[:], start=True, stop=True)
                nc.vector.tensor_tensor(
                    h[:, half * HALF:(half + 1) * HALF, :], hp[:, :, :],
                    _bmid(pb_sb[:, :], HALF), op=ADD)

        def layer(g, l, h):
            at = ats[g]
            # ---- LN stats: mean on GPSIMD, square on ACT, ssq on DVE ----
            musum = st_pool.tile([P, NT], f32, tag="musum")
            nc.vector.tensor_reduce(musum[:], h[:, :, :], axis=AX, op=ADD)
            sqh = tmp_pool.tile([P, NT, H], f32, tag="sqh")
            nc.scalar.activation(sqh[:, :, :], h[:, :, :], AF.Square)
            ssq = st_pool.tile([P, NT], f32, tag="ssq")
            nc.vector.tensor_reduce(ssq[:], sqh[:, :, :], axis=AX, op=ADD)
            mu = st_pool.tile([P, NT], f32, tag="mu")
            nc.vector.tensor_scalar(mu[:], musum[:], 1.0 / H, None, MUL)
            var = st_pool.tile([P, NT], f32, tag="var")
            nc.vector.tensor_tensor(var[:], mu[:], mu[:], op=MUL)
            nc.vector.tensor_scalar(ssq[:], ssq[:], 1.0 / H, None, MUL)
            nc.vector.tensor_tensor(var[:], ssq[:], var[:], op=SUB)
            stdv = st_pool.tile([P, NT], f32, tag="stdv")
            nc.scalar.activation(stdv[:], var[:], AF.Sqrt, bias=eps_sb[:, 0:1])
            rstd = st_pool.tile([P, NT], f32, tag="rstd")
            nc.vector.reciprocal(rstd[:], stdv[:])

            zf = tmp_pool.tile([P, NT, H], f32, tag="zf")
            nc.vector.tensor_tensor(
                zf[:, :, :], h[:, :, :], mu[:].broadcast_to([P, NT, H]), op=SUB)
            if general_ln:
                nc.vector.tensor_tensor(
                    zf[:, :, :], zf[:, :, :],
                    rstd[:].broadcast_to([P, NT, H]), op=MUL)
                nc.vector.tensor_tensor(
                    zf[:, :, :], zf[:, :, :], _bmid(lng_sb[:, l, :], NT), op=MUL)
                nc.vector.tensor_tensor(
                    zf[:, :, :], zf[:, :, :], _bmid(lnb_sb[:, l, :], NT), op=ADD)
                nc.vector.tensor_tensor(
                    zf[:, :, :], zf[:, :, :],
                    dv_sb[:, g, :].broadcast_to([P, NT, H]), op=MUL)
            else:
                rd = st_pool.tile([P, NT], f32, tag="rd")
                nc.vector.tensor_tensor(rd[:], rstd[:], dv_sb[:, g, :], op=MUL)
                nc.vector.tensor_tensor(
                    zf[:, :, :], zf[:, :, :],
                    rd[:].broadcast_to([P, NT, H]), op=MUL)
            # z' = relu(zf) in bf16 (ACT), scz = sc2 * z' (GPSIMD, off DVE)
            zp = z_pool.tile([P, NT, H], bf16)
            nc.scalar.activation(zp[:, :, :], zf[:, :, :], AF.Relu)
            scz = scz_pool.tile([P, NT, H], bf16)
            nc.gpsimd.tensor_tensor(
                scz[:, :, :], zp[:, :, :],
                sc2_sb[:, g, :].broadcast_to([P, NT, H]), op=MUL)

            # ---- aggregation: X^T accumulated in PSUM [128, 512] x NB ----
            xsb = xsb_pool.tile([P, NB, 512], bf16)
            for b in range(NB):
                ps = ps_x.tile([P, 512], f32, tag="xps")
                for jb in range(NT):
                    tgt = ps[0:64, :] if jb % 2 == 0 else ps[64:128, :]
                    nc.tensor.matmul(
                        tgt, lhsT=zp[:, jb, :],
                        rhs=at[:, jb, 512 * b:512 * (b + 1)],
                        start=(jb < 2), stop=(jb == NT - 1))
                for it in range(4):
                    i = 4 * b + it
                    nc.tensor.matmul(
                        ps[0:64, 128 * it:128 * (it + 1)],
                        lhsT=scz[:, i, :], rhs=ident_bf[:],
                        start=False, stop=(it == 3))
                nc.vector.tensor_copy(xsb[:, b, :], ps[:, :])

            # ---- h += dinv9 * (X^T.T @ [W~; W~]) ----
            for half in range(2):
                hp = ps_h.tile([P, HALF, H], f32, tag="hps")
                for q in range(HALF):
                    i = half * HALF + q
                    b, it = divmod(i, 4)
                    nc.tensor.matmul(
                        hp[:, q, :],
                        lhsT=xsb[:, b, 128 * it:128 * (it + 1)],
                        rhs=w2_sb[:, l, :], start=True, stop=True)
                hs = slice(half * HALF, (half + 1) * HALF)
                up = tmp_pool.tile([P, HALF, H], f32, tag="up")
                nc.vector.tensor_tensor(
                    up[:, :, :], hp[:, :, :],
                    dv9_sb[:, g, hs].broadcast_to([P, HALF, H]), op=MUL)
                nc.vector.tensor_tensor(
                    h[:, hs, :], h[:, hs, :], up[:, :, :], op=ADD)

        def head(g, h):
            th = tmp_pool.tile([P, NT, H], f32, tag="zf")
            nc.vector.tensor_tensor(
                th[:, :, :], h[:, :, :], _bmid(hw_sb[:, :], NT), op=MUL)
            osb = ob_pool.tile([P, NT], f32)
            nc.vector.tensor_reduce(osb[:], th[:, :, :], axis=AX, op=ADD)
            nc.vector.tensor_scalar(osb[:], osb[:], hb_sb[:, 0:1], None, ADD)
            nc.sync.dma_start(
                out=out[g].rearrange("(ib p) one -> p (ib one)", p=P),
                in_=osb[:, :])

        # software-pipelined pairs: (0,1) then (2,3); DMAs prefetch ahead
        load_graph(0)
        load_graph(1)
        hs_t = {}
        for pair in range(G // 2):
            g0, g1 = 2 * pair, 2 * pair + 1
            for g in (g0, g1):
                hs_t[g] = h_pool.tile([P, NT, H], f32, name="h", tag="h")
                proj(g, hs_t[g])
            if 2 * pair + 2 < G:
                load_graph(2 * pair + 2)
            if 2 * pair + 3 < G:
                load_graph(2 * pair + 3)
            for l in range(L):
                for g in (g0, g1):
                    layer(g, l, hs_t[g])
            for g in (g0, g1):
                head(g, hs_t[g])

    nc.compile()
    return nc


_NC = {}


def _get_nc(general_ln=False):
    if general_ln not in _NC:
        _NC[general_ln] = _build_nc(general_ln)
    return _NC[general_ln]


def _prep_in_maps(inputs):
    x = np.asarray(inputs["x"], np.float32)
    adj = np.asarray(inputs["adj"], np.float32)
    proj_w = np.asarray(inputs["proj_w"], np.float32)
    proj_b = np.asarray(inputs["proj_b"], np.float32)
    ln_g = np.asarray(inputs["ln_g"], np.float32)
    ln_b = np.asarray(inputs["ln_b"], np.float32)
    conv_w = np.asarray(inputs["conv_w"], np.float32)
    head_w = np.asarray(inputs["head_w"], np.float32)
    head_b = np.asarray(inputs["head_b"], np.float32)

    general_ln = not (
        np.all(ln_g == 1.0) and np.all(ln_b == 0.0))

    wtilde = np.empty((L, H, H), np.float32)
    for l in range(L):
        beta = math.log(LAMDA / (l + 1) + 1.0)
        wtilde[l] = (1.0 - beta) * np.eye(H, dtype=np.float32) + beta * conv_w[l]
    w2 = np.concatenate([wtilde, wtilde], axis=1)  # [L, 128, 64]

    # A^T as fp8 (0/1 exact): bool -> fp8 bitpattern 0x38 == 1.0
    nz = adj != 0                       # [BS, N, N] bool
    a8 = nz.astype(np.uint8) * np.uint8(0x38)
    deg = nz.sum(axis=2, dtype=np.int32).astype(np.float32)   # row sums [BS, N]
    dinv = 1.0 / np.sqrt(deg + 1.0)
    dinv9 = 0.9 * dinv
    sc2 = 1.0 + (deg + 1.0) / 9.0
    # device layout [p, t] with node i = t*128 + p
    dinv_d = dinv.reshape(BS, NT, P).transpose(0, 2, 1)
    dinv9_d = dinv9.reshape(BS, NT, P).transpose(0, 2, 1)
    sc2_d = sc2.reshape(BS, NT, P).transpose(0, 2, 1)

    shared = {
        "pw": proj_w.astype(ml_dtypes.bfloat16),
        "pb": np.ascontiguousarray(np.broadcast_to(proj_b[None, :], (P, H))),
        "w2": w2.astype(ml_dtypes.bfloat16),
        "lng": np.ascontiguousarray(np.broadcast_to(ln_g[:, None, :], (L, P, H))),
        "lnb": np.ascontiguousarray(np.broadcast_to(ln_b[:, None, :], (L, P, H))),
        "hw": np.ascontiguousarray(np.broadcast_to(head_w[:, 0][None, :], (P, H))),
        "hb": np.full((P, 1), float(head_b[0]), np.float32),
    }
    in_maps = []
    for c in range(NCORES):
        sl = slice(c * G, (c + 1) * G)
        in_maps.append(dict(
            shared,
            bt=np.ascontiguousarray(
                a8[sl].transpose(0, 2, 1)).view(ml_dtypes.float8_e4m3),
            xt=np.ascontiguousarray(
                x[sl].transpose(0, 2, 1)).astype(ml_dtypes.bfloat16),
            dv=np.ascontiguousarray(dinv_d[sl]),
            dv9=np.ascontiguousarray(dinv9_d[sl]),
            sc2=np.ascontiguousarray(sc2_d[sl]),
        ))
    return in_maps, general_ln


def kernel(**inputs) -> np.ndarray:
    in_maps, general_ln = _prep_in_maps(inputs)
    nc = _get_nc(general_ln)
    res = run_bass_kernel_spmd(nc, in_maps, list(range(NCORES)))
    return np.concatenate([res.results[c]["out"] for c in range(NCORES)], axis=0)
